# revision 2
# baseline (speedup 1.0000x reference)
"""Multi-head causal attention (B=4, T=2048, C=1024, H=16) on 8 NeuronCores.

Sharding: core i handles batch b = i//2 and head-group g = i%2 (8 heads).
Each core computes QKV projections for its heads, causal attention, and a
row-shard of the output projection (rows g*512:(g+1)*512 of Wp).  The two
partial projection outputs per batch are summed on the host (the tensor-
parallel all-reduce).

Device layout (per core, everything bf16 except PSUM/f32 staging):
  xT  [C=1024, T=2048]    x[b].T, host-pre-transposed
  wq/wk/wv [1024, 512]    per-head [C, HD] stacked along the free dim
  wp  [512, 1024]         row-shard of Wp
  bias [128, 1024] f32    bp broadcast (zeros on g=1 cores)
  mask [128, 4*512] bf16  causal 0/1 masks for the 4 diagonal offsets

Compute (per core):
  QT = wq.T @ xT   -> [512, 2048]   (d on partitions; head-pairs per 128)
  KT = wk.T @ xT   -> [512, 2048]
  V  = xT.T @ wv   -> [2048, 512]
  per head pair p, per tq-block j (512 wide), per tk-tile i (<= 4j+3):
    S^T  = KT_h.T @ QT_h      row-packed 2 heads (K=64 each)
    P    = exp(S^T / 8)       ScalarE, fused scale; diag blocks masked by
                              multiply with 0/1 mask
    den += ones.T @ P         col-packed, replicated across 64 partitions
    OT  += V_h.T @ P          col-packed 2 heads
  OTn = OT * 1/den            -> [512, 2048] bf16
  out = OTn.T @ wp + bias     -> [2048, 1024] f32
"""

import numpy as np
import ml_dtypes

import concourse.bacc as bacc
import concourse.bass as bass
import concourse.tile as tile
import concourse.mybir as mybir
from concourse import bass_utils

BF16 = mybir.dt.bfloat16
F32 = mybir.dt.float32

B, T, C, H = 4, 2048, 1024, 16
HD = 64
N_CORES = 8
HPC = H // 2          # 8 heads per core
DC = HPC * HD         # 512: per-core concat width
NPAIR = HPC // 2      # 4 head pairs
NC_T = T // 128       # 16 tk tiles
TQB = 512
NJ = T // TQB         # 4 tq blocks
NCT = C // 128        # 8 c tiles

_CACHE = {}


def _emit(nc, aps):
    xT, wq, wk, wv, wp, biasb, maskb, out = (
        aps["xT"], aps["wq"], aps["wk"], aps["wv"], aps["wp"],
        aps["bias"], aps["mask"], aps["out"],
    )

    with tile.TileContext(nc) as tc:
        with (
            tc.tile_pool(name="res", bufs=1) as res,
            tc.tile_pool(name="work", bufs=6) as work,
            tc.tile_pool(name="stage", bufs=3) as stage,
            tc.tile_pool(name="mm_ps", bufs=2, space="PSUM") as mm_ps,
            tc.tile_pool(name="s_ps", bufs=4, space="PSUM") as s_ps,
            tc.tile_pool(name="acc_ps", bufs=1, space="PSUM") as acc_ps,
        ):
            # ---- resident loads ----
            xt = []
            for ci in range(NCT):
                t = res.tile([128, T], BF16, tag=f"xt{ci}")
                nc.sync.dma_start(t[:], xT[ci * 128:(ci + 1) * 128, :])
                xt.append(t)
            wq_t, wk_t, wv_t = [], [], []
            for ci in range(NCT):
                for lst, src, nm in ((wq_t, wq, "wq"), (wk_t, wk, "wk"), (wv_t, wv, "wv")):
                    t = res.tile([128, DC], BF16, tag=f"{nm}{ci}")
                    nc.sync.dma_start(t[:], src[ci * 128:(ci + 1) * 128, :])
                    lst.append(t)
            wp_t = []
            for ci in range(DC // 128):
                t = res.tile([128, C], BF16, tag=f"wp{ci}")
                nc.sync.dma_start(t[:], wp[ci * 128:(ci + 1) * 128, :])
                wp_t.append(t)
            bias_t = res.tile([128, C], F32, tag="bias")
            nc.sync.dma_start(bias_t[:], biasb[:, :])
            mask_t = res.tile([128, 4 * TQB], BF16, tag="mask")
            nc.sync.dma_start(mask_t[:], maskb[:, :])
            ones_t = res.tile([128, 64], BF16, tag="ones")
            nc.vector.memset(ones_t[:], 1.0)

            # ---- QT / KT projections: out [d 128 (head pair), tq 512] ----
            qt, kt = [], []
            for p in range(NPAIR):
                tq_ = res.tile([128, T], BF16, tag=f"qt{p}")
                tk_ = res.tile([128, T], BF16, tag=f"kt{p}")
                qt.append(tq_)
                kt.append(tk_)
            for p in range(NPAIR):
                for j in range(NJ):
                    for dst, wt in ((qt, wq_t), (kt, wk_t)):
                        ps = mm_ps.tile([128, TQB], F32, tag="ps")
                        for ci in range(NCT):
                            nc.tensor.matmul(
                                ps[:],
                                wt[ci][:, p * 128:(p + 1) * 128],
                                xt[ci][:, j * TQB:(j + 1) * TQB],
                                start=(ci == 0), stop=(ci == NCT - 1),
                            )
                        nc.vector.tensor_copy(dst[p][:, j * TQB:(j + 1) * TQB], ps[:])

            # ---- V: out [tk 128, d 512] ----
            v_t = []
            for i in range(NC_T):
                vt = res.tile([128, DC], BF16, tag=f"v{i}")
                ps = mm_ps.tile([128, DC], F32, tag="ps")
                for ci in range(NCT):
                    nc.tensor.matmul(
                        ps[:],
                        xt[ci][:, i * 128:(i + 1) * 128],
                        wv_t[ci][:],
                        start=(ci == 0), stop=(ci == NCT - 1),
                    )
                nc.vector.tensor_copy(vt[:], ps[:])
                v_t.append(vt)

            # ---- attention + projection, per tq block ----
            ot = []
            for p in range(NPAIR):
                ot.append(res.tile([128, T], BF16, tag=f"ot{p}", name=f"ot{p}"))

            for j in range(NJ):
                ntk = min(4 * j + 4, NC_T)
                for p in range(NPAIR):
                    ot_ps = acc_ps.tile([128, TQB], F32, tag="ot")
                    den_ps = acc_ps.tile([128, TQB], F32, tag="den")
                    for i in range(ntk):
                        sA = s_ps.tile([128, TQB], F32, tag="s")
                        sB = s_ps.tile([128, TQB], F32, tag="s")
                        nc.tensor.matmul(
                            sA[:],
                            kt[p][0:64, i * 128:(i + 1) * 128],
                            qt[p][0:64, j * TQB:(j + 1) * TQB],
                            start=True, stop=True,
                        )
                        nc.tensor.matmul(
                            sB[:],
                            kt[p][64:128, i * 128:(i + 1) * 128],
                            qt[p][64:128, j * TQB:(j + 1) * TQB],
                            start=True, stop=True,
                        )
                        pA = work.tile([128, TQB], BF16, tag="p")
                        pB = work.tile([128, TQB], BF16, tag="p")
                        nc.scalar.activation(
                            pA[:], sA[:], mybir.ActivationFunctionType.Exp, scale=0.125)
                        nc.scalar.activation(
                            pB[:], sB[:], mybir.ActivationFunctionType.Exp, scale=0.125)
                        oi = i - 4 * j
                        if oi >= 0:  # diagonal block: zero the causally-masked region
                            m = mask_t[:, oi * TQB:(oi + 1) * TQB]
                            nc.vector.tensor_mul(pA[:], pA[:], m)
                            nc.vector.tensor_mul(pB[:], pB[:], m)
                        st, sp = (i == 0), (i == ntk - 1)
                        nc.tensor.matmul(den_ps[0:64, :], ones_t[:], pA[:], start=st, stop=sp)
                        nc.tensor.matmul(den_ps[64:128, :], ones_t[:], pB[:], start=st, stop=sp)
                        nc.tensor.matmul(
                            ot_ps[0:64, :], v_t[i][:, p * 128:p * 128 + 64], pA[:],
                            start=st, stop=sp)
                        nc.tensor.matmul(
                            ot_ps[64:128, :], v_t[i][:, p * 128 + 64:(p + 1) * 128], pB[:],
                            start=st, stop=sp)
                    rec = work.tile([128, TQB], F32, tag="rec")
                    nc.vector.reciprocal(rec[:], den_ps[:])
                    nc.vector.tensor_mul(
                        ot[p][:, j * TQB:(j + 1) * TQB], ot_ps[:], rec[:])

                # projection for this tq block: tq tiles 4j..4j+3
                for jt in range(4 * j, 4 * j + 4):
                    for nh in range(2):
                        ps = mm_ps.tile([128, 512], F32, tag="ps")
                        for cp in range(DC // 128):
                            nc.tensor.matmul(
                                ps[:],
                                ot[cp][:, jt * 128:(jt + 1) * 128],
                                wp_t[cp][:, nh * 512:(nh + 1) * 512],
                                start=(cp == 0), stop=(cp == DC // 128 - 1),
                            )
                        so = stage.tile([128, 512], F32, tag="out")
                        nc.vector.tensor_add(so[:], ps[:], bias_t[:, nh * 512:(nh + 1) * 512])
                        nc.sync.dma_start(
                            out[jt * 128:(jt + 1) * 128, nh * 512:(nh + 1) * 512], so[:])


def _build():
    if "nc" in _CACHE:
        return _CACHE["nc"]
    nc = bacc.Bacc("TRN2", target_bir_lowering=False, debug=False,
                   num_devices=N_CORES)
    aps = {
        "xT": nc.dram_tensor("xT", [C, T], BF16, kind="ExternalInput").ap(),
        "wq": nc.dram_tensor("wq", [C, DC], BF16, kind="ExternalInput").ap(),
        "wk": nc.dram_tensor("wk", [C, DC], BF16, kind="ExternalInput").ap(),
        "wv": nc.dram_tensor("wv", [C, DC], BF16, kind="ExternalInput").ap(),
        "wp": nc.dram_tensor("wp", [DC, C], BF16, kind="ExternalInput").ap(),
        "bias": nc.dram_tensor("bias", [128, C], F32, kind="ExternalInput").ap(),
        "mask": nc.dram_tensor("mask", [128, 4 * TQB], BF16, kind="ExternalInput").ap(),
        "out": nc.dram_tensor("out", [T, C], F32, kind="ExternalOutput").ap(),
    }
    _emit(nc, aps)
    nc.compile()
    _CACHE["nc"] = nc
    return nc


def make_in_maps(x, Wq, Wk, Wv, Wp, bp):
    bf = ml_dtypes.bfloat16
    # causal 0/1 masks for the 4 diagonal offsets o = 0,128,256,384:
    # keep (=1) where col >= o + p
    cols = np.arange(TQB)[None, :]
    rows = np.arange(128)[:, None]
    mask = np.concatenate(
        [(cols >= (o * 128 + rows)).astype(bf) for o in range(4)], axis=1)

    xts = [np.ascontiguousarray(x[b].T).astype(bf) for b in range(B)]
    wqs, wks, wvs, wps, biases = [], [], [], [], []
    for g in range(2):
        hs = slice(g * HPC, (g + 1) * HPC)
        # [HPC, C, HD] -> [C, HPC*HD]
        wqs.append(np.ascontiguousarray(
            Wq[hs].transpose(1, 0, 2).reshape(C, DC)).astype(bf))
        wks.append(np.ascontiguousarray(
            Wk[hs].transpose(1, 0, 2).reshape(C, DC)).astype(bf))
        wvs.append(np.ascontiguousarray(
            Wv[hs].transpose(1, 0, 2).reshape(C, DC)).astype(bf))
        wps.append(np.ascontiguousarray(Wp[g * DC:(g + 1) * DC, :]).astype(bf))
        if g == 0:
            biases.append(np.broadcast_to(
                bp.astype(np.float32), (128, C)).copy())
        else:
            biases.append(np.zeros((128, C), np.float32))

    in_maps = []
    for i in range(N_CORES):
        b, g = i // 2, i % 2
        in_maps.append({
            "xT": xts[b], "wq": wqs[g], "wk": wks[g], "wv": wvs[g],
            "wp": wps[g], "bias": biases[g], "mask": mask,
        })
    return in_maps


def kernel(x, Wq, Wk, Wv, Wp, bp):
    x = np.asarray(x, np.float32)
    Wq, Wk, Wv = (np.asarray(a, np.float32) for a in (Wq, Wk, Wv))
    Wp = np.asarray(Wp, np.float32)
    bp = np.asarray(bp, np.float32)

    nc = _build()
    in_maps = make_in_maps(x, Wq, Wk, Wv, Wp, bp)
    res = bass_utils.run_bass_kernel_spmd(
        nc, in_maps, core_ids=list(range(N_CORES)))
    out = np.empty((B, T, C), np.float32)
    for b in range(B):
        out[b] = res.results[2 * b]["out"] + res.results[2 * b + 1]["out"]
    return out
